# revision 1
# baseline (speedup 1.0000x reference)
"""ChannelAttention TRN2 Bass kernel.

Data-parallel over batch: 8 images, one per NeuronCore. Each core runs the
full pipeline for its [192, 128, 128] image:

  qkv1 = W_qkv @ x (1x1 conv, PE bf16)            [576, hw]
  qkv  = depthwise3x3(qkv1)  (PSUM-accumulated diag matmuls, 9 taps)
  q,k  -> per-row PE transposes -> gram psum accumulation  attn_raw[c,d]
  v    -> stored bf16
  xd   = depthwise3x3(x) -> var over hw (diag bias for attn logits)
  attn = softmax(attn_raw * invq * invk * temp + diag(resc*var))
  y    = (W_proj @ A_blockdiag) @ v   (fold of attn@v and the 1x1 proj)

Channels live on partitions; images are stored 130x130 zero-padded so all
9 depthwise taps are pure free-dim AP offsets.

Partition packing: the two 64-channel tails share one [128, PADN] tile --
v channels 512:576 on partitions 0:64 (unpadded), x channels 128:192 on
partitions 64:128 (padded plane). All matmuls touching the x tail use
operands based at partition 64 on both sides.
"""

import sys

sys.path.insert(0, "/opt/trn_rl_repo")

import numpy as np
import ml_dtypes

BF16 = ml_dtypes.bfloat16

C = 192
H = W = 128
HW = H * W
HP = WP = 130
PADN = HP * WP  # 16900
O3 = 576
HEADS = 8
CHD = 24
NCORES = 8
BANDS = 8
BR = 16  # image rows per band
RB = BR + 2  # band buffer rows (halo)
BANDN = RB * WP  # 2340
CONV_CHUNK = 468  # 2340 / 5
L0P = 120  # heads 0-4 on partitions 0:120
L1P = 72  # heads 5-7

CT_X = [(0, 128), (128, 192)]
CT_O = [(0, 128), (128, 256), (256, 384), (384, 512), (512, 576)]
# taps: t = ky*3+kx, flat offset in padded coords
TAPS = [((t // 3) - 1) * WP + ((t % 3) - 1) for t in range(9)]


def _emit(nc, tc, pools, T, debug, stages=11):
    import concourse.mybir as mybir

    dt = mybir.dt
    Alu = mybir.AluOpType
    Act = mybir.ActivationFunctionType
    AxX = mybir.AxisListType.X

    (pers, scr) = pools

    # ---------------- persistent sbuf tiles ----------------
    xp0 = pers.tile([128, PADN], dt.bfloat16, tag="xp0", name="xp0")
    # packed tail tile: [0:64] = x channels 128:192 (padded plane);
    #                   [64:128] = v channels (dev order) 448:512 (unpadded)
    tail = pers.tile([128, PADN], dt.bfloat16, tag="tail", name="tail")
    v0 = pers.tile([128, HW], dt.bfloat16, tag="v0", name="v0")
    xp1 = tail[0:64, :]
    v1 = tail[64:128, 0:HW]

    wqkvT0 = pers.tile([128, O3], dt.bfloat16, tag="wqkvT0", name="wqkvT0")
    wqkvT1 = pers.tile([64, O3], dt.bfloat16, tag="wqkvT1", name="wqkvT1")
    eye = pers.tile([128, 128], dt.bfloat16, tag="eye", name="eye")
    wdw_sb = pers.tile([128, 18], dt.float32, tag="wdw", name="wdw")  # x dw: tile0 cols 0:9, tile1 cols 9:18 (rows 0:64)
    wodw_sb = pers.tile([128, 45], dt.float32, tag="wodw", name="wodw")  # qkv dw: 5 tiles x 9
    wpt0 = pers.tile([L0P, 192], dt.bfloat16, tag="wpt0", name="wpt0")
    wpt1 = pers.tile([L1P, 192], dt.bfloat16, tag="wpt1", name="wpt1")
    tq0 = pers.tile([L0P, 1], dt.float32, tag="tq0", name="tq0")
    tq1 = pers.tile([L1P, 1], dt.float32, tag="tq1", name="tq1")
    rsc0 = pers.tile([L0P, 1], dt.float32, tag="rsc0", name="rsc0")
    rsc1 = pers.tile([L1P, 1], dt.float32, tag="rsc1", name="rsc1")
    dmask0 = pers.tile([L0P, L0P], dt.float32, tag="dmask0", name="dmask0")
    dmask1 = pers.tile([L1P, L1P], dt.float32, tag="dmask1", name="dmask1")
    maskB0 = pers.tile([L0P, L0P], dt.float32, tag="maskB0", name="maskB0")
    maskB1 = pers.tile([L1P, L1P], dt.float32, tag="maskB1", name="maskB1")
    onescol = pers.tile([1, 128], dt.float32, tag="onescol", name="onescol")

    # diag weight matrices for depthwise taps (bf16)
    # x tile1 diag lives on partitions 64:128
    diag_x0 = [pers.tile([128, 128], dt.bfloat16, tag=f"dgx0_{t}", name=f"dgx0_{t}") for t in range(9)]
    diag_x1 = [pers.tile([64, 64], dt.bfloat16, tag=f"dgx1_{t}", name=f"dgx1_{t}") for t in range(9)]
    diag_o = [[pers.tile([c1 - c0, c1 - c0], dt.bfloat16, tag=f"dgo{i}_{t}", name=f"dgo{i}_{t}")
               for t in range(9)] for i, (c0, c1) in enumerate(CT_O)]

    qkv1_band = [pers.tile([c1 - c0, BANDN], dt.bfloat16, tag=f"q1b{i}", name=f"q1b{i}")
                 for i, (c0, c1) in enumerate(CT_O)]
    qdw_q0 = pers.tile([128, BR * W], dt.bfloat16, tag="qdw_q0", name="qdw_q0")
    qdw_k0 = pers.tile([128, BR * W], dt.bfloat16, tag="qdw_k0", name="qdw_k0")
    qdw_qb = pers.tile([64, BR * W], dt.bfloat16, tag="qdw_qb", name="qdw_qb")
    qdw_kb = pers.tile([64, BR * W], dt.bfloat16, tag="qdw_kb", name="qdw_kb")

    nsq = [pers.tile([128, BANDS], dt.float32, tag=f"nsq{i}", name=f"nsq{i}") for i in range(4)]
    sxcol0 = pers.tile([128, 48], dt.float32, tag="sxc0", name="sxc0")
    sxsqcol0 = pers.tile([128, 48], dt.float32, tag="sxq0", name="sxq0")
    sxcol1 = pers.tile([64, 48], dt.float32, tag="sxc1", name="sxc1")
    sxsqcol1 = pers.tile([64, 48], dt.float32, tag="sxq1", name="sxq1")

    # ---------------- load inputs ----------------
    xp0_3d = xp0[:].rearrange("p (a b) -> p a b", a=HP)
    xp1_3d = xp1.rearrange("p (a b) -> p a b", a=HP)

    # zero only the pad cells: rows 0 and 129, cols 0 and 129
    for tgt in (xp0[:], tail[0:64, :]):
        t3 = tgt.rearrange("p (a b) -> p a b", a=HP)
        nc.vector.memset(t3[:, 0, :], 0.0)
        nc.vector.memset(t3[:, 129, :], 0.0)
        nc.vector.memset(t3[:, 1:129, 0], 0.0)
        nc.vector.memset(t3[:, 1:129, 129], 0.0)

    xin = T["x"]
    nc.sync.dma_start(
        out=xp0_3d[:, 1:129, 1:129],
        in_=xin[0:128, :].rearrange("p (a b) -> p a b", a=H),
    )
    nc.sync.dma_start(
        out=xp1_3d[:, 1:129, 1:129],
        in_=xin[128:192, :].rearrange("p (a b) -> p a b", a=H),
    )
    nc.sync.dma_start(out=wqkvT0[:], in_=T["wqkvT"][0:128, :])
    nc.sync.dma_start(out=wqkvT1[:], in_=T["wqkvT"][128:192, :])
    nc.sync.dma_start(out=eye[:], in_=T["eye"][:])
    nc.sync.dma_start(out=wdw_sb[0:128, 0:9], in_=T["wdw"][0:128, :])
    nc.sync.dma_start(out=wdw_sb[0:64, 9:18], in_=T["wdw"][128:192, :])
    for i, (c0, c1) in enumerate(CT_O):
        nc.sync.dma_start(out=wodw_sb[0:c1 - c0, 9 * i:9 * i + 9],
                          in_=T["wodw"][c0:c1, :])
    nc.sync.dma_start(out=wpt0[:], in_=T["wpt"][0:L0P, :])
    nc.sync.dma_start(out=wpt1[:], in_=T["wpt"][L0P:C, :])
    nc.sync.dma_start(out=tq0[:], in_=T["tempvec"][0:L0P, :])
    nc.sync.dma_start(out=tq1[:], in_=T["tempvec"][L0P:C, :])
    nc.sync.dma_start(out=rsc0[:], in_=T["rescvec"][0:L0P, :])
    nc.sync.dma_start(out=rsc1[:], in_=T["rescvec"][L0P:C, :])
    nc.sync.dma_start(out=dmask0[:], in_=T["dmask0"][:])
    nc.sync.dma_start(out=dmask1[:], in_=T["dmask1"][:])
    nc.sync.dma_start(out=maskB0[:], in_=T["maskB0"][:])
    nc.sync.dma_start(out=maskB1[:], in_=T["maskB1"][:])
    nc.vector.memset(onescol[:], 1.0)

    # diag(w_tap) = eye * w_col (per-partition scalar)
    for t in range(9):
        nc.vector.tensor_scalar(diag_x0[t][:], eye[:], wdw_sb[:, t:t + 1],
                                None, Alu.mult)
        nc.vector.tensor_scalar(diag_x1[t][:], eye[0:64, 0:64],
                                wdw_sb[0:64, 9 + t:9 + t + 1], None, Alu.mult)
    for i, (c0, c1) in enumerate(CT_O):
        cw = c1 - c0
        for t in range(9):
            nc.vector.tensor_scalar(diag_o[i][t][:], eye[0:cw, 0:cw],
                                    wodw_sb[0:cw, 9 * i + t:9 * i + t + 1],
                                    None, Alu.mult)

    def xp_flat(i, lo, hi):
        return xp0[:, lo:hi] if i == 0 else tail[0:64, lo:hi]

    # gram psum banks (one accumulation group per bank -- start=True
    # zeroes the whole 2KB bank)
    gram0 = scr["gram"].tile([128, 512], dt.float32, tag="gram0", name="gram0")
    gram1 = scr["gram"].tile([128, 512], dt.float32, tag="gram1", name="gram1")

    def evac(out_ap, in_ap, eng="v"):
        if eng == "v":
            nc.vector.tensor_copy(out_ap, in_ap)
        else:
            nc.scalar.copy(out_ap, in_ap)

    # ---------------- bands: conv -> dw -> transpose/gram ----------------
    for b in range(BANDS if stages >= 1 else 0):
        band_off = (BR * b) * WP
        # 1x1 conv over band (18 rows x 130 cols, padded layout)
        for oc, (o0, o1) in enumerate(CT_O):
            ow = o1 - o0
            for cc in range(5):
                lo = band_off + cc * CONV_CHUNK
                ps = scr["conv"].tile([128, CONV_CHUNK], dt.float32, tag="conv", name="conv")
                nc.tensor.matmul(
                    ps[0:ow, :], wqkvT0[:, o0:o1], xp_flat(0, lo, lo + CONV_CHUNK),
                    start=True, stop=False, skip_group_check=True)
                nc.tensor.matmul(
                    ps[0:ow, :], wqkvT1[:, o0:o1],
                    xp_flat(1, lo, lo + CONV_CHUNK),
                    start=False, stop=True, skip_group_check=True)
                evac(qkv1_band[oc][:, cc * CONV_CHUNK:(cc + 1) * CONV_CHUNK],
                     ps[0:ow, :], "a")

        # depthwise 3x3 on qkv1 band: 9 diag matmuls accumulated in psum
        dw_groups = [(0, 3), (3, 3), (6, 3), (9, 3), (12, 3), (15, 1)]
        for oc, (o0, o1) in enumerate(CT_O if stages >= 2 else []):
            ow = o1 - o0
            for (r0, nr) in dw_groups:
                start_f = (1 + r0) * WP + 1
                ln = nr * WP - 2
                ps = scr["dw"].tile([128, 390], dt.float32, tag="dw", name="dw")
                for t in range(9):
                    nc.tensor.matmul(
                        ps[0:ow, 0:ln], diag_o[oc][t][:],
                        qkv1_band[oc][:, start_f + TAPS[t]:start_f + TAPS[t] + ln],
                        start=(t == 0), stop=(t == 8), skip_group_check=True)
                # interior rows at 130-stride, 128 cols each
                src_int = ps[0:ow, 0:nr * WP].rearrange(
                    "p (a b) -> p a b", a=nr, b=WP)[:, :, 0:W]
                gpos = (BR * b + r0) * W
                d0 = r0 * W
                nw = nr * W
                if oc == 0 or oc == 1:
                    dstt = qdw_q0 if oc == 0 else qdw_k0
                    evac(dstt[:, d0:d0 + nw].rearrange("p (a b) -> p a b", a=nr),
                         src_int)
                elif oc == 2:
                    evac(v0[:, gpos:gpos + nw].rearrange("p (a b) -> p a b", a=nr),
                         src_int)
                elif oc == 3:
                    evac(qdw_qb[:, d0:d0 + nw].rearrange("p (a b) -> p a b", a=nr),
                         src_int[0:64])
                    evac(v1[:, gpos:gpos + nw].rearrange("p (a b) -> p a b", a=nr),
                         src_int[64:128])
                else:
                    evac(qdw_kb[:, d0:d0 + nw].rearrange("p (a b) -> p a b", a=nr),
                         src_int[0:64])

        # norms accumulation (sum of squares over band)
        if stages >= 3:
            for i, (tile_, cw) in enumerate([(qdw_q0, 128), (qdw_k0, 128),
                                             (qdw_qb, 64), (qdw_kb, 64)]):
                s = scr["ttr"].tile([128, BR * W], dt.bfloat16, tag="ttr", name="ttr")
                nc.vector.scalar_tensor_tensor(
                    s[0:cw, :], tile_[:], 1.0, tile_[:],
                    Alu.mult, Alu.mult, accum_out=nsq[i][0:cw, b:b + 1])

        # per-row transposes + gram accumulation
        for r in range(BR if stages >= 4 else 0):
            ps_t = scr["tp"].tile([128, 448], dt.bfloat16, tag="tp", name="tp")
            rr = r * W
            nc.tensor.matmul(ps_t[0:128, 0:128], qdw_q0[:, rr:rr + W],
                             eye[:], is_transpose=True,
                             start=True, stop=False, skip_group_check=True)
            nc.tensor.matmul(ps_t[0:128, 128:192], qdw_qb[:, rr:rr + W],
                             eye[0:64, 0:64], is_transpose=True,
                             start=False, stop=False, skip_group_check=True)
            nc.tensor.matmul(ps_t[0:128, 224:352], qdw_k0[:, rr:rr + W],
                             eye[:], is_transpose=True,
                             start=False, stop=False, skip_group_check=True)
            nc.tensor.matmul(ps_t[0:128, 352:416], qdw_kb[:, rr:rr + W],
                             eye[0:64, 0:64], is_transpose=True,
                             start=False, stop=True, skip_group_check=True)
            qt = scr["qt"].tile([128, 192], dt.bfloat16, tag="qt", name="qt")
            kt = scr["qt"].tile([128, 192], dt.bfloat16, tag="kt", name="kt")
            if stages >= 5:
                evac(qt[:], ps_t[:, 0:192])
                evac(kt[:], ps_t[:, 224:416])
            else:
                nc.vector.tensor_copy(qt[:], ps_t[:, 0:192])
                nc.vector.tensor_copy(kt[:], ps_t[:, 224:416])
            if stages >= 6:
                first = (b == 0 and r == 0)
                last = (b == BANDS - 1 and r == BR - 1)
                nc.tensor.matmul(gram0[0:L0P, 0:L0P], qt[:, 0:L0P], kt[:, 0:L0P],
                                 start=first, stop=last, skip_group_check=True)
                nc.tensor.matmul(gram1[0:L1P, 0:L1P], qt[:, L0P:192],
                                 kt[:, L0P:192],
                                 start=first, stop=last, skip_group_check=True)

    # ---------------- x depthwise -> variance ----------------
    nrows = [3] * 42 + [2]
    for i in range(2 if stages >= 7 else 0):
        cw = 128 if i == 0 else 64
        sxc = sxcol0 if i == 0 else sxcol1
        sxq = sxsqcol0 if i == 0 else sxsqcol1
        row = 0
        for j, nr in enumerate(nrows):
            start_f = (1 + row) * WP + 1
            ln = nr * WP - 2
            ps = scr["conv"].tile([128, CONV_CHUNK], dt.float32, tag="conv", name="conv")
            for t in range(9):
                lhsT = diag_x0[t][:] if i == 0 else diag_x1[t][:]
                nc.tensor.matmul(
                    ps[0:cw, 0:ln], lhsT,
                    xp_flat(i, start_f + TAPS[t], start_f + TAPS[t] + ln),
                    start=(t == 0), stop=(t == 8), skip_group_check=True)
            src_int = ps[0:cw, 0:nr * WP].rearrange("p (a b) -> p a b", a=nr, b=WP)
            sA = scr["xs"].tile([128, 3 * W], dt.float32, tag="xsA", name="xsA")
            sB = scr["xs"].tile([128, 3 * W], dt.float32, tag="xsB", name="xsB")
            sA3 = sA[0:cw, 0:nr * W].rearrange("p (a b) -> p a b", a=nr)
            sB3 = sB[0:cw, 0:nr * W].rearrange("p (a b) -> p a b", a=nr)
            nc.vector.tensor_scalar(
                sA3, src_int[:, :, 0:W], 1.0, None, Alu.mult, Alu.add,
                accum_out=sxc[0:cw, j:j + 1])
            nc.scalar.activation(
                sB3, src_int[:, :, 0:W], Act.Square,
                accum_out=sxq[0:cw, j:j + 1])
            row += nr

    # var = (Sxx - Sx^2/HW) / (HW-1)
    vart = []
    for i in range(2 if stages >= 7 else 0):
        cw = 128 if i == 0 else 64
        sxc = sxcol0 if i == 0 else sxcol1
        sxq = sxsqcol0 if i == 0 else sxsqcol1
        sx = pers.tile([cw, 1], dt.float32, tag=f"sx{i}", name=f"sx{i}")
        sxx = pers.tile([cw, 1], dt.float32, tag=f"sxx{i}", name=f"sxx{i}")
        nc.vector.tensor_reduce(sx[:], sxc[0:cw, 0:43], AxX, Alu.add)
        nc.vector.tensor_reduce(sxx[:], sxq[0:cw, 0:43], AxX, Alu.add)
        sx2 = pers.tile([cw, 1], dt.float32, tag=f"sx2{i}", name=f"sx2{i}")
        nc.vector.tensor_tensor(sx2[:], sx[:], sx[:], Alu.mult)
        nc.vector.tensor_scalar(sx2[:], sx2[:], -1.0 / (HW * (HW - 1)), None,
                                Alu.mult)
        var_i = pers.tile([cw, 1], dt.float32, tag=f"var{i}", name=f"var{i}")
        nc.vector.scalar_tensor_tensor(
            var_i[:], sxx[:], 1.0 / (HW - 1), sx2[:], Alu.mult, Alu.add)
        vart.append(var_i)

    if stages < 10:
        return
    # ---------------- norms -> inverse norms ----------------
    nred = []
    for i, cw in enumerate([128, 128, 64, 64]):
        t = pers.tile([cw, 1], dt.float32, tag=f"nred{i}", name=f"nred{i}")
        nc.vector.tensor_reduce(t[:], nsq[i][0:cw, :], AxX, Alu.add)
        nred.append(t)
    # nred[0]=q 0:128, nred[1]=k 0:128, nred[2]=q 128:192, nred[3]=k 128:192

    # assemble L1-group (q channels 120:192) via sbuf->sbuf DMAs
    sq_l1 = pers.tile([L1P, 1], dt.float32, tag="sql1", name="sql1")
    nc.gpsimd.dma_start(out=sq_l1[0:8, :], in_=nred[0][120:128, :])
    nc.gpsimd.dma_start(out=sq_l1[8:72, :], in_=nred[2][:, :])
    bias_l1 = pers.tile([L1P, 1], dt.float32, tag="biasl1", name="biasl1")
    nc.gpsimd.dma_start(out=bias_l1[0:8, :], in_=vart[0][120:128, :])
    nc.gpsimd.dma_start(out=bias_l1[8:72, :], in_=vart[1][0:64, :])

    def rsqrt(dst, src, tagp, base=0):
        # dst = 1/sqrt(src) with one Newton step after ACT Sqrt.
        # Scratch lives at the same partition base as src (SB two-input ops
        # require equal base partitions).
        w = src.shape[0]
        fcols = src.shape[-1] if len(src.shape) > 1 else 1
        def mk(nm):
            t = pers.tile([base + w, fcols], dt.float32,
                          tag=f"rs_{nm}{tagp}", name=f"rs_{nm}{tagp}")
            return t[base:base + w, :]
        r, y0, t1 = mk("r"), mk("y"), mk("t")
        nc.vector.reciprocal(r, src)
        nc.scalar.activation(y0, r, Act.Sqrt)
        nc.vector.tensor_tensor(t1, y0, y0, Alu.mult)
        nc.vector.tensor_tensor(t1, t1, src, Alu.mult)
        nc.vector.tensor_scalar(t1, t1, -0.5, 1.5, Alu.mult, Alu.add)
        nc.vector.tensor_tensor(dst, y0, t1, Alu.mult)

    invq0 = pers.tile([L0P, 1], dt.float32, tag="invq0", name="invq0")
    invq1 = pers.tile([L1P, 1], dt.float32, tag="invq1", name="invq1")
    rsqrt(invq0[:], nred[0][0:L0P, :], 0)
    rsqrt(invq1[:], sq_l1[:], 1)
    nc.vector.tensor_tensor(invq0[:], invq0[:], tq0[:], Alu.mult)
    nc.vector.tensor_tensor(invq1[:], invq1[:], tq1[:], Alu.mult)

    # invk in k-channel column layouts: k 0:128 (nred[1]), k 128:192 (nred[3])
    invkA = pers.tile([128, 1], dt.float32, tag="invkA", name="invkA")
    invkB = pers.tile([64, 1], dt.float32, tag="invkB", name="invkB")
    rsqrt(invkA[:], nred[1][:, :], "2a")
    rsqrt(invkB[:], nred[3][:, :], "2b")
    # assemble invk as rows (partition->free remap needs DMA)
    invkrow0 = pers.tile([1, L0P], dt.float32, tag="invkrow0", name="invkrow0")
    invkrow1 = pers.tile([1, L1P], dt.float32, tag="invkrow1", name="invkrow1")
    nc.gpsimd.dma_start(out=invkrow0[0:1, 0:120], in_=invkA[0:120, :])
    nc.gpsimd.dma_start(out=invkrow1[0:1, 0:8], in_=invkA[120:128, :])
    nc.gpsimd.dma_start(out=invkrow1[0:1, 8:72], in_=invkB[:, :])

    if stages < 9:
        return
    # ---------------- softmax per L group ----------------
    A_bd = [pers.tile([L0P, 192], dt.bfloat16, tag="abd0", name="abd0"),
            pers.tile([L1P, 192], dt.bfloat16, tag="abd1", name="abd1")]
    nc.vector.memset(A_bd[0][:], 0.0)
    nc.vector.memset(A_bd[1][:], 0.0)

    # evacuate gram psum to sbuf (psum APs must start at partition 0/32/64/96)
    Gsb0 = pers.tile([L0P, L0P], dt.float32, tag="Gsb0", name="Gsb0")
    Gsb1 = pers.tile([L1P, L1P], dt.float32, tag="Gsb1", name="Gsb1")
    nc.vector.tensor_copy(Gsb0[:], gram0[0:L0P, 0:L0P])
    nc.vector.tensor_copy(Gsb1[:], gram1[0:L1P, 0:L1P])

    dbg_L = []
    for g, (gw, nheads, h0) in enumerate([(L0P, 5, 0), (L1P, 3, 5)]):
        invq = invq0 if g == 0 else invq1
        dmask = dmask0 if g == 0 else dmask1
        maskB = maskB0 if g == 0 else maskB1
        invkrow = invkrow0 if g == 0 else invkrow1
        Gsb = Gsb0 if g == 0 else Gsb1
        rsc = rsc0 if g == 0 else rsc1
        biascol = vart[0][0:L0P, :] if g == 0 else bias_l1[:]
        # invk broadcast down partitions: K=1 outer product ones x invkrow
        ps_bt = scr["tp"].tile([128, 448], dt.float32, tag="tp", name="ps_bt")
        ps_b = ps_bt[0:gw, 0:gw]
        nc.tensor.matmul(ps_b, onescol[0:1, 0:gw], invkrow[0:1, 0:gw],
                         start=True, stop=True, skip_group_check=True)
        L2 = pers.tile([gw, gw], dt.float32, tag=f"L2{g}", name=f"L2{g}")
        nc.vector.tensor_scalar(L2[:], Gsb[:], invq[:], None, Alu.mult)
        nc.vector.tensor_tensor(L2[:], L2[:], ps_b, Alu.mult)
        # off-block cells -> -1e30; diag bias = dmask * (resc*var)
        nc.vector.tensor_tensor(L2[:], L2[:], maskB[:], Alu.add)
        bcol = pers.tile([gw, 1], dt.float32, tag=f"bcol{g}", name=f"bcol{g}")
        nc.vector.tensor_tensor(bcol[:], biascol, rsc[:], Alu.mult)
        L3 = pers.tile([gw, gw], dt.float32, tag=f"L3{g}", name=f"L3{g}")
        nc.vector.scalar_tensor_tensor(L3[:], dmask[:], bcol[:], L2[:],
                                       Alu.mult, Alu.add)
        nm = pers.tile([gw, 1], dt.float32, tag=f"nm{g}", name=f"nm{g}")
        nc.vector.tensor_reduce(nm[:], L3[:], AxX, Alu.max)
        nc.vector.tensor_scalar(nm[:], nm[:], -1.0, None, Alu.mult)
        E = pers.tile([gw, gw], dt.float32, tag=f"E{g}", name=f"E{g}")
        ssum = pers.tile([gw, 1], dt.float32, tag=f"ss{g}", name=f"ss{g}")
        nc.scalar.activation(E[:], L3[:], Act.Exp, bias=nm[:], accum_out=ssum[:])
        rinv = pers.tile([gw, 1], dt.float32, tag=f"ri{g}", name=f"ri{g}")
        nc.vector.reciprocal(rinv[:], ssum[:])
        cbase = 0 if g == 0 else L0P
        nc.vector.tensor_scalar(A_bd[g][:, cbase:cbase + gw], E[:], rinv[:],
                                None, Alu.mult)
        dbg_L.append(L3)

    # ---------------- WpA^T = A_bd^T @ Wp^T ----------------
    ps_w0t = scr["conv"].tile([128, 512], dt.float32, tag="conv", name="ps_w0t")
    ps_w1t = scr["dw"].tile([128, 512], dt.float32, tag="dw", name="ps_w1t")
    ps_w0 = ps_w0t[0:128, 0:192]
    ps_w1 = ps_w1t[64:128, 0:192]
    for g, (abd, wpt) in enumerate([(A_bd[0], wpt0), (A_bd[1], wpt1)]):
        nc.tensor.matmul(ps_w0, abd[:, 0:128], wpt[:],
                         start=(g == 0), stop=(g == 1), skip_group_check=True)
        nc.tensor.matmul(ps_w1, abd[:, 128:192], wpt[:],
                         start=(g == 0), stop=(g == 1), skip_group_check=True)
    wpat0 = pers.tile([128, 192], dt.bfloat16, tag="wpat0", name="wpat0")
    wpat1 = pers.tile([128, 192], dt.bfloat16, tag="wpat1", name="wpat1")
    nc.vector.tensor_copy(wpat0[:], ps_w0)
    nc.vector.tensor_copy(wpat1[64:128, :], ps_w1)

    if stages < 10:
        return
    # ---------------- final y = (WpA) @ v ----------------
    yout = T["y"]
    for nch in range(32):
        lo = nch * 512
        psy0 = scr["conv"].tile([128, 512], dt.float32, tag="conv", name="psy0")
        psy1t = scr["dw"].tile([128, 512], dt.float32, tag="dw", name="psy1")
        psy1 = psy1t[0:64, :]
        nc.tensor.matmul(psy0[:], wpat0[:, 0:128], v0[:, lo:lo + 512],
                         start=True, stop=False, skip_group_check=True)
        nc.tensor.matmul(psy0[:], wpat1[64:128, 0:128], v1[:, lo:lo + 512],
                         start=False, stop=True, skip_group_check=True)
        nc.tensor.matmul(psy1, wpat0[:, 128:192], v0[:, lo:lo + 512],
                         start=True, stop=False, skip_group_check=True)
        nc.tensor.matmul(psy1, wpat1[64:128, 128:192], v1[:, lo:lo + 512],
                         start=False, stop=True, skip_group_check=True)
        yb0 = scr["yb"].tile([128, 512], dt.float32, tag="yb0", name="yb0")
        yb1 = scr["yb"].tile([64, 512], dt.float32, tag="yb1", name="yb1")
        evac(yb0[:], psy0[:], "a")
        evac(yb1[:], psy1, "a")
        nc.sync.dma_start(out=yout[0:128, lo:lo + 512], in_=yb0[:])
        nc.sync.dma_start(out=yout[128:192, lo:lo + 512], in_=yb1[:])

    # ---------------- debug dumps ----------------
    if debug and stages >= 11:
        nc.sync.dma_start(out=T["d_var"][0:128, :], in_=vart[0][:])
        nc.sync.dma_start(out=T["d_var"][128:192, :], in_=vart[1][:])
        for i, cw in enumerate([128, 128, 64, 64]):
            nc.sync.dma_start(out=T["d_nsq"][128 * i:128 * i + cw, :],
                              in_=nred[i][:])
        for oc in range(5):
            cw = CT_O[oc][1] - CT_O[oc][0]
            nc.sync.dma_start(out=T["d_qkv1"][128 * oc:128 * oc + cw, :],
                              in_=qkv1_band[oc][:])
        for i, (tile_, cw) in enumerate([(qdw_q0, 128), (qdw_k0, 128),
                                         (qdw_qb, 64), (qdw_kb, 64)]):
            nc.sync.dma_start(out=T["d_qdw"][128 * i:128 * i + cw, :],
                              in_=tile_[:])
        nc.sync.dma_start(out=T["d_v"][0:128, :], in_=v0[:])
        nc.sync.dma_start(out=T["d_v"][128:192, :], in_=v1)
        nc.sync.dma_start(out=T["d_abd"][0:L0P, :], in_=A_bd[0][:])
        nc.sync.dma_start(out=T["d_abd"][L0P:C, :], in_=A_bd[1][:])
        nc.sync.dma_start(out=T["d_wpat"][0:128, :], in_=wpat0[:])
        nc.sync.dma_start(out=T["d_wpat"][128:192, :], in_=wpat1[64:128, :])



def build_nc(n_iters=1, debug=False, stages=11):
    import concourse.bacc as bacc
    import concourse.mybir as mybir
    from concourse import tile as tile_mod

    dt = mybir.dt
    nc = bacc.Bacc("TRN2", target_bir_lowering=False, debug=False)

    T = {}
    T["x"] = nc.dram_tensor("x", [C, HW], dt.bfloat16, kind="ExternalInput")
    T["wqkvT"] = nc.dram_tensor("wqkvT", [C, O3], dt.bfloat16, kind="ExternalInput")
    T["wdw"] = nc.dram_tensor("wdw", [C, 9], dt.float32, kind="ExternalInput")
    T["wodw"] = nc.dram_tensor("wodw", [O3, 9], dt.float32, kind="ExternalInput")
    T["eye"] = nc.dram_tensor("eye", [128, 128], dt.bfloat16, kind="ExternalInput")
    T["wpt"] = nc.dram_tensor("wpt", [C, C], dt.bfloat16, kind="ExternalInput")
    T["tempvec"] = nc.dram_tensor("tempvec", [C, 1], dt.float32, kind="ExternalInput")
    T["rescvec"] = nc.dram_tensor("rescvec", [C, 1], dt.float32, kind="ExternalInput")
    T["dmask0"] = nc.dram_tensor("dmask0", [L0P, L0P], dt.float32, kind="ExternalInput")
    T["dmask1"] = nc.dram_tensor("dmask1", [L1P, L1P], dt.float32, kind="ExternalInput")
    T["maskB0"] = nc.dram_tensor("maskB0", [L0P, L0P], dt.float32, kind="ExternalInput")
    T["maskB1"] = nc.dram_tensor("maskB1", [L1P, L1P], dt.float32, kind="ExternalInput")
    T["y"] = nc.dram_tensor("y", [C, HW], dt.float32, kind="ExternalOutput")
    if debug:
        T["d_var"] = nc.dram_tensor("d_var", [C, 1], dt.float32, kind="ExternalOutput")
        T["d_nsq"] = nc.dram_tensor("d_nsq", [512, 1], dt.float32, kind="ExternalOutput")
        T["d_qkv1"] = nc.dram_tensor("d_qkv1", [640, BANDN], dt.bfloat16, kind="ExternalOutput")
        T["d_qdw"] = nc.dram_tensor("d_qdw", [512, BR * W], dt.bfloat16, kind="ExternalOutput")
        T["d_v"] = nc.dram_tensor("d_v", [C, HW], dt.bfloat16, kind="ExternalOutput")
        T["d_abd"] = nc.dram_tensor("d_abd", [C, C], dt.bfloat16, kind="ExternalOutput")
        T["d_wpat"] = nc.dram_tensor("d_wpat", [C, C], dt.bfloat16, kind="ExternalOutput")

    with tile_mod.TileContext(nc) as tc:
        with tc.tile_pool(name="pers", bufs=1) as pers, \
             tc.tile_pool(name="ps_gram", bufs=1, space="PSUM") as ps_gram, \
             tc.tile_pool(name="ps_conv", bufs=2, space="PSUM") as ps_conv, \
             tc.tile_pool(name="ps_dw", bufs=2, space="PSUM") as ps_dw, \
             tc.tile_pool(name="ps_tp", bufs=2, space="PSUM") as ps_tp, \
             tc.tile_pool(name="scr_ttr", bufs=2) as scr_ttr, \
             tc.tile_pool(name="scr_qt", bufs=3) as scr_qt, \
             tc.tile_pool(name="scr_xs", bufs=2) as scr_xs, \
             tc.tile_pool(name="scr_yb", bufs=4) as scr_yb:
            scr = {"gram": ps_gram, "conv": ps_conv, "dw": ps_dw, "tp": ps_tp,
                   "ttr": scr_ttr, "qt": scr_qt, "xs": scr_xs, "yb": scr_yb}
            pools = (pers, scr)
            if n_iters > 1:
                with tc.For_i(0, n_iters, 1):
                    _emit(nc, tc, pools, T, debug, stages)
            else:
                _emit(nc, tc, pools, T, debug, stages)
    nc.compile()
    return nc


def dev_channel_perm():
    """Device qkv channel order -> original channel index.

    dev 0:128   = q 0:128    (orig 0:128)
    dev 128:256 = k 0:128    (orig 192:320)
    dev 256:384 = v 0:128    (orig 384:512)
    dev 384:448 = q 128:192  (orig 128:192)
    dev 448:512 = v 128:192  (orig 512:576)
    dev 512:576 = k 128:192  (orig 320:384)
    """
    return np.concatenate([
        np.arange(0, 128), np.arange(192, 320), np.arange(384, 512),
        np.arange(128, 192), np.arange(512, 576), np.arange(320, 384),
    ])


def host_prep(inputs):
    """Build per-core in_maps from full inputs."""
    x = np.asarray(inputs["x"])
    w_dw = np.asarray(inputs["w_dw"])
    w_qkv = np.asarray(inputs["w_qkv"])
    w_qkvdw = np.asarray(inputs["w_qkvdw"])
    w_proj = np.asarray(inputs["w_proj"])
    temperature = np.asarray(inputs["temperature"])
    rescale = np.asarray(inputs["rescale"])

    perm = dev_channel_perm()
    wqkvT = np.ascontiguousarray(w_qkv[:, :, 0, 0].T[:, perm]).astype(BF16)
    wdw9 = w_dw[:, 0].reshape(C, 9).astype(np.float32)
    wodw9 = w_qkvdw[:, 0].reshape(O3, 9)[perm].astype(np.float32)
    wpt = np.ascontiguousarray(w_proj[:, :, 0, 0].T).astype(BF16)
    t8 = temperature[:, 0, 0].astype(np.float32)
    r8 = rescale[:, 0].astype(np.float32)
    tempvec = np.repeat(t8, CHD)[:, None].astype(np.float32)
    rescvec = np.repeat(r8, CHD)[:, None].astype(np.float32)
    eye = np.eye(128, dtype=BF16)
    dmask0 = np.eye(L0P, dtype=np.float32)
    dmask1 = np.eye(L1P, dtype=np.float32)
    blk0 = np.zeros((L0P, L0P), np.float32) - 1e30
    for h in range(5):
        blk0[24 * h:24 * h + CHD, 24 * h:24 * h + CHD] = 0.0
    blk1 = np.zeros((L1P, L1P), np.float32) - 1e30
    for h in range(3):
        blk1[24 * h:24 * h + CHD, 24 * h:24 * h + CHD] = 0.0

    shared = {
        "wqkvT": wqkvT, "wdw": wdw9, "wodw": wodw9, "eye": eye, "wpt": wpt,
        "tempvec": tempvec, "rescvec": rescvec, "dmask0": dmask0,
        "dmask1": dmask1, "maskB0": blk0, "maskB1": blk1,
    }
    in_maps = []
    for i in range(NCORES):
        m = dict(shared)
        m["x"] = np.ascontiguousarray(x[i].reshape(C, HW)).astype(BF16)
        in_maps.append(m)
    return in_maps


_BUILT = {}


def get_built(n_iters=1, debug=False):
    key = (n_iters, debug)
    if key not in _BUILT:
        _BUILT[key] = build_nc(n_iters=n_iters, debug=debug)
    return _BUILT[key]


def run_on_hw(nc, in_maps):
    from concourse.bass_utils import run_bass_kernel_spmd
    res = run_bass_kernel_spmd(nc, in_maps, list(range(NCORES)))
    return res.results


def kernel(**inputs):
    nc = get_built()
    in_maps = host_prep(inputs)
    results = run_on_hw(nc, in_maps)
    y = np.stack([results[i]["y"].reshape(C, H, W) for i in range(NCORES)])
    return y.astype(np.float32)

